# revision 19
# baseline (speedup 1.0000x reference)
"""Multi-head causal self-attention (S=4096, D=2048, H=16) on 8 trn2 NeuronCores.

Sharding: tensor-parallel over heads — 2 heads per core. Each core computes
q/k/v projections for its head group, causal attention, and its partial
out-projection; the host sums the 8 partials (the "all-reduce").

v4 strategy (per core), tuned so the PE never idles (TRN2 PE p-states make
every gap cost double — the 2.4 GHz clock needs ~3us of continuous busy):
  - all matmul inputs bf16 (same 1 cycle/row PE stream rate as fp32r, half
    the DMA/SBUF traffic); PSUM accumulation fp32.
  - host pre-transposes x -> xT [D, S]; projections produce qT/kT in [hd, s]
    layout and v natural [s, hd]; no on-device transposes anywhere.
  - attention runs on 256-wide sq chunks: finer causal granularity (~6% less
    score work than 512) and every score tile is exactly one PSUM bank.
  - PSUM sub-bank packing via the has_written-bit semantics: the first
    matmul into a bank carries start=True (clears the bank's bits), later
    matmuls into disjoint regions overwrite (bit unset) and accumulating
    ones add (bit set). This packs v-projection PSUM into 2 banks (double
    buffered) and ps_o+ps_d into one shared bank.
  - scores computed transposed: sT[sk, sq] = k @ qT; one [128,512] exp per
    sk-pair on ScalarE (scale folded in), bf16 out.
  - softmax denominator: ones-matmul with an all-ones [128,128] stationary
    accumulated next to ps_o in the same bank — the row-sum lands broadcast
    on all 128 partitions, so normalization is reciprocal_approx_fast + one
    vector multiply, nothing cross-partition.
  - the diagonal (masked) pair's QK is emitted first but its PV/DEN run
    last, so the exp->mask chain never gates the PE; QK emission runs 2-3
    pairs ahead of PV (score pipeline depth 3 + the diagonal bank).
  - PSUM drains are spread across VectorE/ScalarE; out-projection emission
    is delayed into the next chunk and its copies alternate engines.
  - biases: bq/bk during projection drains (per-partition scalars); bv via
    host-side rank-1 correction (softmax rows sum to 1); bo on host.
    Partials returned bf16, summed on host in fp32.
"""

import numpy as np
import ml_dtypes

S, D, H = 4096, 2048, 16
HD = D // H  # 128
N_CORES = 8
HPC = H // N_CORES  # heads per core = 2
DPC = HPC * HD  # head dims per core = 256
SCALE = 1.0 / np.sqrt(np.float32(HD))

SQ = 512  # phase-1 sq chunk width
NSQ = S // SQ  # 8
SQ2 = 256  # phase-2 sq chunk width
NQC = S // SQ2  # 16
NKT = S // 128  # 32 sk tiles
NDT = D // 128  # 16 d tiles

_CACHE = {}


def _build(reps: int = 1):
    import concourse.bacc as bacc
    import concourse.mybir as mybir
    import concourse.tile as tile

    f32 = mybir.dt.float32
    bf = mybir.dt.bfloat16

    nc = bacc.Bacc("TRN2", target_bir_lowering=False)

    xT = nc.dram_tensor("xT", [D, S], bf, kind="ExternalInput")
    wq = nc.dram_tensor("wq", [D, DPC], bf, kind="ExternalInput")
    wk = nc.dram_tensor("wk", [D, DPC], bf, kind="ExternalInput")
    wv = nc.dram_tensor("wv", [D, DPC], bf, kind="ExternalInput")
    wo = nc.dram_tensor("wo", [DPC, D], bf, kind="ExternalInput")
    bqk = nc.dram_tensor("bqk", [2, DPC], f32, kind="ExternalInput")
    masks = nc.dram_tensor("masks", [128, 128], bf, kind="ExternalInput")
    out = nc.dram_tensor("out", [S, D], bf, kind="ExternalOutput")

    xT3 = xT.rearrange("(dt p) s -> p dt s", p=128)
    out3 = out.rearrange("(st p) d -> p st d", p=128)

    with tile.TileContext(nc) as tc:
        with (
            tc.tile_pool(name="persist", bufs=1) as persist,
            tc.tile_pool(name="misc", bufs=1) as misc,
        ):
            # persistent SBUF tensors
            qT = persist.tile([128, HPC, S], bf, tag="qT")
            kT = persist.tile([128, HPC, S], bf, tag="kT")
            vn = persist.tile([128, NKT, DPC], bf, tag="vn")  # v natural [sk, hd]
            wo_sb = persist.tile([128, HPC, D], bf, tag="wo")
            mask_sb = persist.tile([128, 128], bf, tag="mask")
            bias_sb = misc.tile([128, 2, HPC], f32, tag="bias")  # [.,0,.]=bq [.,1,.]=bk
            ones_bf = misc.tile([128, 128], bf, tag="ones_bf")

            nc.vector.memset(ones_bf[:], 1.0)

            if reps == 1:
                _phases(nc, tc, mybir, f32, bf, qT, kT, vn, wo_sb, mask_sb,
                        bias_sb, ones_bf, xT3, out3, wq, wk, wv, wo, bqk, masks)
            else:
                with tc.For_i(0, reps, 1):
                    _phases(nc, tc, mybir, f32, bf, qT, kT, vn, wo_sb, mask_sb,
                            bias_sb, ones_bf, xT3, out3, wq, wk, wv, wo, bqk,
                            masks)
    nc.finalize()
    return nc


def _phases(nc, tc, mybir, f32, bf, qT, kT, vn, wo_sb, mask_sb, bias_sb,
            ones_bf, xT3, out3, wq, wk, wv, wo, bqk, masks):
    Exp = mybir.ActivationFunctionType.Exp
    Copy = mybir.ActivationFunctionType.Copy
    Identity = mybir.ActivationFunctionType.Identity

    # ---------------- Phase 1: projections ----------------
    with (
        tc.tile_pool(name="wproj", bufs=1) as wproj,
        tc.tile_pool(name="xin", bufs=3) as xin,
        tc.tile_pool(name="psqk", bufs=1, space="PSUM") as psqk,
        tc.tile_pool(name="psv", bufs=2, space="PSUM") as psv,
    ):
        wq_sb = wproj.tile([128, NDT, DPC], bf, tag="wq")
        wk_sb = wproj.tile([128, NDT, DPC], bf, tag="wk")
        wv_sb = wproj.tile([128, NDT, DPC], bf, tag="wv")

        # DMA emission order = need order, quarter-granular at the head so
        # the first matmuls start as early as possible. Phase-2-only
        # tensors (mask, wo) go last.
        xts = {}
        xts[(0, 0)] = xin.tile([128, NDT // 2, SQ], bf, tag="xt", name="xt00")
        for lo, hi in ((0, 2), (2, 4), (4, 8)):
            nc.sync.dma_start(
                out=xts[(0, 0)][:, lo:hi, :], in_=xT3[:, lo:hi, 0:SQ]
            )
            for w_sb, w_dram in ((wk_sb, wk), (wq_sb, wq), (wv_sb, wv)):
                nc.sync.dma_start(
                    out=w_sb[:, lo:hi, :],
                    in_=w_dram.rearrange("(dt p) m -> p dt m", p=128)[:, lo:hi, :],
                )
        nc.sync.dma_start(
            out=bias_sb[:], in_=bqk.rearrange("b (h p) -> p b h", p=128)
        )
        xts[(0, 1)] = xin.tile([128, NDT // 2, SQ], bf, tag="xt", name="xt01")
        nc.sync.dma_start(out=xts[(0, 1)][:], in_=xT3[:, 8:16, 0:SQ])
        for w_sb, w_dram in ((wk_sb, wk), (wq_sb, wq), (wv_sb, wv)):
            nc.sync.dma_start(
                out=w_sb[:, 8:16, :],
                in_=w_dram.rearrange("(dt p) m -> p dt m", p=128)[:, 8:16, :],
            )
        nc.sync.dma_start(out=mask_sb[:], in_=masks[:, :])
        nc.sync.dma_start(out=wo_sb[:], in_=wo.rearrange("(h p) d -> p h d", p=128))

        for j in range(NSQ):
            ps_q = psqk.tile([128, HPC, SQ], f32, tag="psq")
            ps_k = psqk.tile([128, HPC, SQ], f32, tag="psk")
            # 4 v accumulation regions packed into 2 banks (sub-bank groups)
            ps_v = psv.tile([128, 4, DPC], f32, tag="psv")

            def kkqq(xt, dt):
                st = dict(start=(dt == 0), stop=(dt == NDT - 1))
                for ps, w_sb in ((ps_k, wk_sb), (ps_q, wq_sb)):
                    for h in range(HPC):
                        nc.tensor.matmul(
                            ps[:, h, :],
                            w_sb[:, dt, h * 128 : h * 128 + 128],
                            xt[:, dt % 8, :],
                            **st,
                        )

            def vmm(xt, dt):
                for i in range(4):
                    # start only on the first matmul of each bank (i=0 -> bank
                    # 0, i=2 -> bank 1); regions i=1/i=3 rely on overwrite-
                    # when-bit-unset
                    nc.tensor.matmul(
                        ps_v[:, i, :],
                        xt[:, dt % 8, i * 128 : i * 128 + 128],
                        wv_sb[:, dt, :],
                        start=(dt == 0 and i % 2 == 0),
                        stop=(dt == NDT - 1),
                        skip_group_check=True,
                    )

            halves = {}
            for half in range(2):
                halves[half] = xts.pop((j, half))
                # prefetch the next xt while this one computes
                nxt = (j, half + 1) if half == 0 else (j + 1, 0)
                if nxt[0] < NSQ and nxt not in xts:
                    xts[nxt] = xin.tile(
                        [128, NDT // 2, SQ], bf, tag="xt", name=f"xt{nxt[0]}{nxt[1]}"
                    )
                    nc.sync.dma_start(
                        out=xts[nxt][:],
                        in_=xT3[
                            :,
                            nxt[1] * 8 : nxt[1] * 8 + 8,
                            nxt[0] * SQ : (nxt[0] + 1) * SQ,
                        ],
                    )
                # v-matmul emission lags one dt so the chunk start is pure
                # k/q (whose PSUM frees first) and the chunk end is v-heavy
                # (k/q groups stop early -> drains overlap the v tail)
                for dtl in range(NDT // 2):
                    dt = half * 8 + dtl
                    kkqq(halves[half], dt)
                    if dt > 0:
                        vmm(halves[(dt - 1) // 8], dt - 1)
                    if dt == NDT - 1:
                        vmm(halves[1], dt)
            sq = slice(j * SQ, (j + 1) * SQ)
            # drains split across VectorE/ScalarE so PSUM frees in parallel
            nc.vector.tensor_scalar_add(kT[:, 0, sq], ps_k[:, 0, :],
                                        bias_sb[:, 1, 0:1])
            nc.scalar.activation(kT[:, 1, sq], ps_k[:, 1, :], Identity,
                                 bias=bias_sb[:, 1, 1:2])
            nc.scalar.activation(qT[:, 0, sq], ps_q[:, 0, :], Identity,
                                 bias=bias_sb[:, 0, 0:1])
            nc.vector.tensor_scalar_add(qT[:, 1, sq], ps_q[:, 1, :],
                                        bias_sb[:, 0, 1:2])
            nc.vector.tensor_copy(vn[:, 4 * j : 4 * j + 4, :], ps_v[:])

    # ---------------- Phase 2: attention + out-proj ----------------
    with (
        tc.tile_pool(name="expp", bufs=4) as expp,
        tc.tile_pool(name="otp", bufs=2) as otp,
        tc.tile_pool(name="outp", bufs=2) as outp,
        tc.tile_pool(name="rdp", bufs=2) as rdp,
        tc.tile_pool(name="pss", bufs=3, space="PSUM") as pss,
        tc.tile_pool(name="psdg", bufs=1, space="PSUM") as psdg,
        tc.tile_pool(name="psod", bufs=2, space="PSUM") as psod,
        tc.tile_pool(name="psb", bufs=2, space="PSUM") as psb,
    ):
        def outproj(jc, oT):
            # out-projection for s-chunk jc (both heads accumulate);
            # PSUM->SBUF copies alternate ScalarE/VectorE
            for n in range(4):
                ob = outp.tile([128, 2, 512], bf, tag="ob")
                for si in range(2):
                    ps_p = psb.tile([128, 512], f32, tag="ps_r")
                    for h in range(HPC):
                        nc.tensor.matmul(
                            ps_p[:],
                            oT[:, h, si * 128 : si * 128 + 128],
                            wo_sb[:, h, n * 512 : n * 512 + 512],
                            start=(h == 0),
                            stop=(h == HPC - 1),
                        )
                    if (n + si) % 2 == 0:
                        nc.scalar.activation(ob[:, si, :], ps_p[:], Copy)
                    else:
                        nc.vector.tensor_copy(ob[:, si, :], ps_p[:])
                    nc.sync.dma_start(
                        out=out3[:, 2 * jc + si, n * 512 : n * 512 + 512],
                        in_=ob[:, si, :],
                    )

        pending = None
        for jc in range(NQC):
            sq = slice(jc * SQ2, (jc + 1) * SQ2)
            npairs = jc + 1
            diag = npairs - 1
            others = list(range(npairs - 1))
            oT = otp.tile([128, HPC, SQ2], bf, tag="oT")
            for h in range(HPC):

                def qk(pi, pool_tile, trim=False):
                    # one sk-pair of scores into one PSUM bank (two disjoint
                    # halves, single bank group). trim: the diagonal pair's
                    # u1 tile only contributes to sq columns 128..255.
                    for u in range(2):
                        cl = 128 if (trim and u == 1) else 0
                        nc.tensor.matmul(
                            pool_tile[:, u, cl:SQ2],
                            kT[:, h, (2 * pi + u) * 128 : (2 * pi + u) * 128 + 128],
                            qT[:, h, jc * SQ2 + cl : (jc + 1) * SQ2],
                            start=(u == 0),
                            stop=(u == 1),
                            skip_group_check=True,
                        )

                def qk_exp(pi):
                    s = pss.tile([128, 2, SQ2], f32, tag="ps_s")
                    qk(pi, s)
                    e = expp.tile([128, 2, SQ2], bf, tag="ex")
                    nc.scalar.activation(e[:], s[:], Exp, scale=float(SCALE))
                    return e

                # ps_o ([:,0,:]) and ps_d ([:,1,:]) share one bank; the very
                # first PV carries start=True (clears the bank), everything
                # else relies on the has_written bits.
                ps_od = psod.tile([128, 2, SQ2], f32, tag="ps_od")

                def pv_den(pi, e, first, last, trim=False):
                    for u in range(2):
                        cl = 128 if (trim and u == 1) else 0
                        nc.tensor.matmul(
                            ps_od[:, 0, cl:SQ2],
                            vn[:, 2 * pi + u, h * 128 : h * 128 + 128],
                            e[:, u, cl:SQ2],
                            start=(first and u == 0),
                            stop=(last and u == 1),
                            skip_group_check=True,
                        )
                    for u in range(2):
                        cl = 128 if (trim and u == 1) else 0
                        nc.tensor.matmul(
                            ps_od[:, 1, cl:SQ2],
                            ones_bf[:],
                            e[:, u, cl:SQ2],
                            start=False,
                            stop=(last and u == 1),
                            skip_group_check=True,
                        )

                # diagonal pair: QK first (own bank), PV/DEN last; u1 only
                # contributes to sq cols 128.. (the rest is fully masked)
                sdg = psdg.tile([128, 2, SQ2], f32, tag="sdg")
                qk(diag, sdg, trim=True)
                exd = None

                def exp_diag():
                    ed = expp.tile([128, 2, SQ2], bf, tag="ex")
                    nc.scalar.activation(ed[:], sdg[:], Exp, scale=float(SCALE))
                    # in-place triangle masks: (u0, cols 0:128) and
                    # (u1, cols 128:256) share the same c>=p pattern
                    nc.vector.tensor_mul(ed[:, 0, 0:128], ed[:, 0, 0:128],
                                         mask_sb[:])
                    nc.vector.tensor_mul(ed[:, 1, 128:SQ2], ed[:, 1, 128:SQ2],
                                         mask_sb[:])
                    return ed

                if npairs == 1:
                    exd = exp_diag()

                exq = {}
                for pi in others[:2]:
                    exq[pi] = qk_exp(pi)

                proc = others + [diag]
                pend_pos = max(0, npairs - 3)
                for i, pi in enumerate(proc):
                    if i + 2 < len(others):
                        exq[others[i + 2]] = qk_exp(others[i + 2])
                    e = exd if pi == diag else exq.pop(pi)
                    pv_den(pi, e, first=(i == 0), last=(i == npairs - 1),
                           trim=(pi == diag))
                    if exd is None and i == pend_pos:
                        exd = exp_diag()
                    if pending is not None and h == 0 and i == min(1, npairs - 1):
                        pending()
                        pending = None
                rd = rdp.tile([128, SQ2], f32, tag="rd")
                nc.vector.reciprocal_approx_fast(rd[:], ps_od[:, 1, :])
                nc.vector.tensor_mul(oT[:, h, :], ps_od[:, 0, :], rd[:])
            pending = (lambda jc=jc, oT=oT: outproj(jc, oT))
        pending()


def _get_nc():
    if "nc" not in _CACHE:
        _CACHE["nc"] = _build()
    return _CACHE["nc"]


def _host_masks() -> np.ndarray:
    # the shared 128x128 causal triangle: keep iff c >= p
    p = np.arange(128)[:, None]
    c = np.arange(128)[None, :]
    return np.ascontiguousarray((c >= p).astype(np.float32))  # [128, 128]


def make_in_maps(inputs: dict) -> list:
    bf = ml_dtypes.bfloat16
    Wq, bq = np.asarray(inputs["Wq"], np.float32), np.asarray(inputs["bq"], np.float32)
    Wk, bk = np.asarray(inputs["Wk"], np.float32), np.asarray(inputs["bk"], np.float32)
    Wv = np.asarray(inputs["Wv"], np.float32)
    Wo = np.asarray(inputs["Wo"], np.float32)
    xT = np.ascontiguousarray(
        np.asarray(inputs["hidden_states"], np.float32).T.astype(bf)
    )
    masks = _host_masks().astype(bf)
    in_maps = []
    for c in range(N_CORES):
        r = slice(c * DPC, (c + 1) * DPC)
        in_maps.append(
            {
                "xT": xT,
                "wq": np.ascontiguousarray(Wq[r, :].T.astype(bf)),
                "wk": np.ascontiguousarray(Wk[r, :].T.astype(bf)),
                "wv": np.ascontiguousarray(Wv[r, :].T.astype(bf)),
                "wo": np.ascontiguousarray(Wo[:, r].T.astype(bf)),
                "bqk": np.stack([bq[r], bk[r]]),
                "masks": masks,
            }
        )
    return in_maps


def kernel(hidden_states, Wq, bq, Wk, bk, Wv, bv, Wo, bo):
    from concourse.bass_utils import run_bass_kernel_spmd

    Wv, bv = np.asarray(Wv, np.float32), np.asarray(bv, np.float32)
    Wo, bo = np.asarray(Wo, np.float32), np.asarray(bo, np.float32)
    in_maps = make_in_maps(
        dict(hidden_states=hidden_states, Wq=Wq, bq=bq, Wk=Wk, bk=bk, Wv=Wv, Wo=Wo)
    )

    nc = _get_nc()
    results = run_bass_kernel_spmd(nc, in_maps, core_ids=list(range(N_CORES))).results

    acc = results[0]["out"].astype(np.float32)
    for c in range(1, N_CORES):
        acc += results[c]["out"].astype(np.float32)
    # bias corrections: bo plus the deferred bv contribution (attn rows sum to 1)
    acc += (bo + bv @ Wo.T)[None, :]
    return acc
